# revision 2
# baseline (speedup 1.0000x reference)
"""Trainium2 Bass kernel: CMAFM fusion (segment min/max stats -> attention
MLPs -> gated 2-layer MLP over voxels), data-parallel over the batch axis.

Sharding: batch b -> NeuronCore b (batch_idx is sorted, B == n_cores == 8).
Each core computes its own batch's feature min/max stats locally, runs the
tiny attention MLPs on-device, folds the per-feature gating into the first
fused-MLP weight matrix, and runs the big MLP over its voxels. No
collectives: every voxel's gating row is core-local by construction.

Design notes:
- Ingest via hardware DMA-transpose (xbar): HBM [chunk,128] bf16 -> SBUF
  [128,chunk] feature-major directly (contiguous 4KB M2S reads). No PE
  transposes, no PSUM staging copies, no host-side permutation.
- Segment min/max: one DVE tensor_reduce per (stream, stat, chunk) into a
  partials buffer; one combine per stat at finalize feeds the tiny MLPs,
  whose attention output is folded into W_f1 as a per-partition scale.
- Fused MLP: L1 streams voxels against the gated W_f1 (N=512 matmuls);
  L2 runs operand-swapped with W_f2 slices stationary (4 distinct
  lhsT loads per rep, N=512) producing feature-major output. h1 relu
  evacuation on Act, output relu evacuation on DVE (PSUM sources cap
  each engine at 1 elem/cycle/lane, so the two streams are split).
- Stores write [2,128,S] feature-major with 2-4KB-contiguous descriptors
  on the SP HWDGE ring; the host transposes back when unsharding.
Weights load once; the rep loop (timing harness) re-runs ingest + stats +
MLP with rep r+1's ingest interleaved into rep r's MLP loop so
consecutive reps pipeline across engines.
"""

import os
import sys

import numpy as np

for _p in ("/opt/trn_rl_repo",):
    if os.path.isdir(_p) and _p not in sys.path:
        sys.path.append(_p)

B = 8
L = 128
C = 128
OUT = 256
CA = 512
H = 170
VT = 512     # voxels per compute tile
CH = 2048    # voxels per ingest/store chunk (tail chunk may be 1024)

STATS_GP = 0  # retained for older harness compatibility (unused)

_cache = {}


def _chunks(S_pad):
    spans = []
    v0 = 0
    while v0 < S_pad:
        w = min(CH, S_pad - v0)
        spans.append((v0, w))
        v0 += w
    return spans


def _build(S_pad, stats_gp=0, reps=1):
    from contextlib import ExitStack

    import concourse.bacc as bacc
    import concourse.mybir as mybir
    import concourse.tile as tile

    f32 = mybir.dt.float32
    bf16 = mybir.dt.bfloat16
    Alu = mybir.AluOpType
    Act = mybir.ActivationFunctionType

    spans = _chunks(S_pad)
    NCH = len(spans)

    nc = bacc.Bacc("TRN2", target_bir_lowering=False, debug=False, num_devices=B)
    lidar = nc.dram_tensor("lidar", [S_pad, L], bf16, kind="ExternalInput").ap()
    cam = nc.dram_tensor("cam", [S_pad, C], bf16, kind="ExternalInput").ap()
    wl1 = nc.dram_tensor("W_l1", [CA, H], f32, kind="ExternalInput").ap()
    wl2 = nc.dram_tensor("W_l2", [H, L], f32, kind="ExternalInput").ap()
    wc1 = nc.dram_tensor("W_c1", [CA, H], f32, kind="ExternalInput").ap()
    wc2 = nc.dram_tensor("W_c2", [H, C], f32, kind="ExternalInput").ap()
    wf1 = nc.dram_tensor("W_f1", [2 * L, OUT], f32, kind="ExternalInput").ap()
    wf2 = nc.dram_tensor("W_f2", [OUT, OUT], f32, kind="ExternalInput").ap()
    out = nc.dram_tensor("out", [2, 128, S_pad], bf16, kind="ExternalOutput").ap()

    with tile.TileContext(nc) as tc, ExitStack() as ctx:
        wpool = ctx.enter_context(tc.tile_pool(name="weights", bufs=1))
        respool = ctx.enter_context(tc.tile_pool(name="res", bufs=1))
        statpool = ctx.enter_context(tc.tile_pool(name="stat", bufs=1))
        h1pool = ctx.enter_context(tc.tile_pool(name="h1", bufs=2))
        xoutpool = ctx.enter_context(tc.tile_pool(name="xout", bufs=2))
        psl1 = ctx.enter_context(tc.tile_pool(name="psl1", bufs=2, space="PSUM"))
        psl2 = ctx.enter_context(tc.tile_pool(name="psl2", bufs=2, space="PSUM"))

        # --- weight SBUF tiles ---
        wf1_s = wpool.tile([128, 2, OUT], f32)
        wf2_stage = wpool.tile([128, 2, OUT], f32)
        wf2_s = wpool.tile([128, 2, OUT], bf16)
        w1e_s = wpool.tile([128, 2, OUT], bf16)
        wl1_s = wpool.tile([128, 4, H], f32)
        wc1_s = wpool.tile([128, 4, H], f32)
        wl2a_s = wpool.tile([128, L], f32)
        wl2b_s = wpool.tile([H - 128, L], f32)
        wc2a_s = wpool.tile([128, C], f32)
        wc2b_s = wpool.tile([H - 128, C], f32)

        nc.sync.dma_start(wl1_s[:], wl1.rearrange("(a p) h -> p a h", p=128))
        nc.sync.dma_start(wc1_s[:], wc1.rearrange("(a p) h -> p a h", p=128))
        nc.sync.dma_start(wl2a_s[:], wl2[0:128, :])
        nc.sync.dma_start(wl2b_s[:], wl2[128:H, :])
        nc.sync.dma_start(wc2a_s[:], wc2[0:128, :])
        nc.sync.dma_start(wc2b_s[:], wc2[128:H, :])
        nc.sync.dma_start(wf1_s[:], wf1.rearrange("(a p) o -> p a o", p=128))
        nc.sync.dma_start(wf2_stage[:], wf2.rearrange("(a p) o -> p a o", p=128))
        nc.scalar.activation(wf2_s[:], wf2_stage[:], Act.Copy)

        # preload activation tables off the critical path
        warm = wpool.tile([128, 1], f32)
        nc.vector.memset(warm[:], 0.0)
        nc.scalar.activation(warm[:], warm[:], Act.Relu)
        nc.scalar.activation(warm[:], warm[:], Act.Sigmoid)

        # resident feature-major voxel data + per-chunk stat partials
        xres = {
            "l": respool.tile([128, S_pad], bf16, name="xres_l", tag="xres_l"),
            "c": respool.tile([128, S_pad], bf16, name="xres_c", tag="xres_c"),
        }
        stream_src = {"l": lidar, "c": cam}
        partials = {}
        for key in ("min_l", "max_l", "min_c", "max_c"):
            partials[key] = statpool.tile([128, NCH], f32, name="p" + key,
                                          tag="p" + key)

        chain_of = {"min_l": 0, "max_l": 1, "min_c": 2, "max_c": 3}
        tiny_ctx = {}

        def emit_ingest_chunk(ci):
            """Transpose-DMA one chunk of both streams + 4 stat reduces."""
            c0, w = spans[ci]
            for which in ("l", "c"):
                nc.sync.dma_start_transpose(
                    xres[which][:, c0 : c0 + w],
                    stream_src[which][c0 : c0 + w, :],
                )
            for which in ("l", "c"):
                for statname, op in (("min", Alu.min), ("max", Alu.max)):
                    key = statname + "_" + which
                    nc.vector.tensor_reduce(
                        partials[key][:, ci : ci + 1],
                        xres[which][:, c0 : c0 + w],
                        axis=mybir.AxisListType.X,
                        op=op,
                    )

        def emit_finalize_tiny():
            """Combine partials -> stats -> tiny MLPs -> gated W_f1 fold."""
            ps_t = psl1.tile([128, 2, VT], f32, name="ps1", tag="psl1")
            tiny_all = ps_t[:, 0, 0:OUT]
            for key in ("min_l", "max_l", "min_c", "max_c"):
                k = chain_of[key]
                op = Alu.min if key.startswith("min") else Alu.max
                s = statpool.tile([128, 1], f32, tag="stat" + key)
                nc.vector.tensor_reduce(
                    s[:], partials[key][:], axis=mybir.AxisListType.X, op=op
                )
                for name, w1_s in (("l", wl1_s), ("c", wc1_s)):
                    for tag, mo, mn in (("h1a", 0, 128), ("h1b", 128, H - 128)):
                        ch = 2 * (name == "c") + (tag == "h1b")
                        nc.tensor.matmul(
                            tiny_all[0:mn, ch * 4 + k : ch * 4 + k + 1],
                            w1_s[:, k, mo : mo + mn],
                            s[:],
                            start=True,
                            stop=True,
                        )

            heads = {"l": (wl2a_s, wl2b_s), "c": (wc2a_s, wc2b_s)}
            hsums = {}
            for name in ("l", "c"):
                for tag, mo, mn in (("h1a", 0, 128), ("h1b", 128, H - 128)):
                    ch = 2 * (name == "c") + (tag == "h1b")
                    hp = statpool.tile([mn, 1], f32, tag=tag + "p" + name)
                    nc.vector.tensor_reduce(
                        hp[:], tiny_all[0:mn, ch * 4 : ch * 4 + 4],
                        axis=mybir.AxisListType.X, op=Alu.add,
                    )
                    hs = statpool.tile([mn, 1], f32, tag=tag + "s" + name)
                    nc.vector.tensor_scalar_max(hs[:], hp[:], 0.0)
                    hsums[name, tag] = hs
            for name in ("l", "c"):
                acol = 16 if name == "l" else 18
                w2a_s, w2b_s = heads[name]
                nc.tensor.matmul(
                    tiny_all[:, acol : acol + 1], w2a_s[:],
                    hsums[name, "h1a"][:], start=True, stop=True,
                )
                nc.tensor.matmul(
                    tiny_all[:, acol + 1 : acol + 2], w2b_s[:],
                    hsums[name, "h1b"][:], start=True, stop=True,
                )
            atts = {}
            for name in ("l", "c"):
                acol = 16 if name == "l" else 18
                att_r = statpool.tile([128, 1], f32, tag="attr" + name)
                nc.vector.tensor_reduce(
                    att_r[:], tiny_all[:, acol : acol + 2],
                    axis=mybir.AxisListType.X, op=Alu.add,
                )
                attp = statpool.tile([128, 1], f32, tag="attp" + name)
                nc.vector.tensor_scalar_max(attp[:], att_r[:], 0.0)
                att = statpool.tile([128, 1], f32, tag="att" + name)
                nc.scalar.activation(att[:], attp[:], Act.Sigmoid)
                atts[name] = att
            nc.scalar.activation(
                w1e_s[:, 0, :], wf1_s[:, 0, :], Act.Copy, scale=atts["l"][:]
            )
            nc.scalar.activation(
                w1e_s[:, 1, :], wf1_s[:, 1, :], Act.Copy, scale=atts["c"][:]
            )

        def emit_l1(t):
            """L1 matmuls for tile t -> [128(hid half), 2, VT] PSUM + relu."""
            xt_l = xres["l"][:, t * VT : (t + 1) * VT]
            xt_c = xres["c"][:, t * VT : (t + 1) * VT]
            ps1 = psl1.tile([128, 2, VT], f32, name="ps1", tag="psl1")
            for m in range(2):
                sl = ps1[:, m, :]
                nc.tensor.matmul(
                    sl, w1e_s[:, 0, m * 128 : (m + 1) * 128], xt_l,
                    start=True, stop=False,
                )
                nc.tensor.matmul(
                    sl, w1e_s[:, 1, m * 128 : (m + 1) * 128], xt_c,
                    start=False, stop=True,
                )
            h1t = h1pool.tile([128, 2, VT], bf16, tag="h1")
            nc.scalar.activation(h1t[:], ps1[:], Act.Relu)
            return h1t

        def emit_l2(t, h1t, xout, o):
            """L2 (W_f2 stationary): [128(out half), 2, VT] PSUM + relu."""
            ps2 = psl2.tile([128, 2, VT], f32, name="ps2", tag="psl2")
            for g in range(2):
                sl = ps2[:, g, :]
                nc.tensor.matmul(
                    sl, wf2_s[:, 0, g * 128 : (g + 1) * 128], h1t[:, 0, :],
                    start=True, stop=False,
                )
                nc.tensor.matmul(
                    sl, wf2_s[:, 1, g * 128 : (g + 1) * 128], h1t[:, 1, :],
                    start=False, stop=True,
                )
            dst = xout[:, :, o : o + VT]
            if t % 2 == 0:
                nc.vector.tensor_scalar_max(dst, ps2[:], 0.0)
            else:
                nc.scalar.activation(dst, ps2[:], Act.Relu)

        # prologue: rep 0's ingest + finalize
        for ci in range(NCH):
            emit_ingest_chunk(ci)
        emit_finalize_tiny()

        for r in range(reps):
            h1cur = emit_l1(0)
            tprev = 0
            oprev = None
            for ci in range(NCH):
                c0, w = spans[ci]
                xout = xoutpool.tile([128, 2, CH], bf16, tag="xout")
                for ti in range(w // VT):
                    t = c0 // VT + ti
                    # software pipeline: issue l1(t+1) before l2(t)
                    nxt = t + 1
                    h1next = emit_l1(nxt) if nxt < S_pad // VT else None
                    emit_l2(t, h1cur, xout, ti * VT)
                    h1cur = h1next
                nc.sync.dma_start(
                    out[:, :, c0 : c0 + w].rearrange("g p s -> p g s"),
                    xout[:, :, 0:w],
                )
                if r + 1 < reps:
                    emit_ingest_chunk(ci)
            if r + 1 < reps:
                emit_finalize_tiny()

    nc.compile()
    return nc


def _get_program(S_pad):
    if S_pad not in _cache:
        _cache[S_pad] = _build(S_pad)
    return _cache[S_pad]


def _to_bf16(a):
    import ml_dtypes

    return np.asarray(a, np.float32).astype(ml_dtypes.bfloat16)


def shard_inputs(lidar, cam, batch_idx, W_l1, W_l2, W_c1, W_c2, W_f1, W_f2):
    """Split by batch (batch_idx sorted), pad with replicated real rows."""
    lidar = _to_bf16(lidar)
    cam = _to_bf16(cam)
    batch_idx = np.asarray(batch_idx)
    bounds = np.searchsorted(batch_idx, np.arange(B + 1))
    sizes = np.diff(bounds)
    S_pad = int(-(-max(int(sizes.max()), 1) // 1024) * 1024)
    weights = {
        "W_l1": np.ascontiguousarray(W_l1, np.float32),
        "W_l2": np.ascontiguousarray(W_l2, np.float32),
        "W_c1": np.ascontiguousarray(W_c1, np.float32),
        "W_c2": np.ascontiguousarray(W_c2, np.float32),
        "W_f1": np.ascontiguousarray(W_f1, np.float32),
        "W_f2": np.ascontiguousarray(W_f2, np.float32),
    }
    in_maps = []
    for b in range(B):
        s0, s1 = int(bounds[b]), int(bounds[b + 1])
        n = s1 - s0
        l = np.empty((S_pad, L), lidar.dtype)
        c = np.empty((S_pad, C), cam.dtype)
        if n > 0:
            l[:n] = lidar[s0:s1]
            c[:n] = cam[s0:s1]
            l[n:] = lidar[s1 - 1]
            c[n:] = cam[s1 - 1]
        else:
            l[:] = 0
            c[:] = 0
        in_maps.append({"lidar": l, "cam": c, **weights})
    return in_maps, bounds, sizes, S_pad


def unshard_one(res_out, S_pad, n):
    """[2,128,S_pad] bf16 feature-major -> [n, 256] f32 voxel-major."""
    r = np.asarray(res_out).reshape(2 * 128, S_pad)
    return r[:, :n].T.astype(np.float32)


def kernel(lidar, cam, batch_idx, W_l1, W_l2, W_c1, W_c2, W_f1, W_f2):
    from concourse.bass_utils import run_bass_kernel_spmd

    in_maps, bounds, sizes, S_pad = shard_inputs(
        lidar, cam, batch_idx, W_l1, W_l2, W_c1, W_c2, W_f1, W_f2
    )
    nc = _get_program(S_pad)
    res = run_bass_kernel_spmd(nc, in_maps, core_ids=list(range(B)))
    N = lidar.shape[0]
    out_full = np.empty((N, OUT), np.float32)
    for b in range(B):
        s0, s1 = int(bounds[b]), int(bounds[b + 1])
        if s1 > s0:
            out_full[s0:s1] = unshard_one(res.results[b]["out"], S_pad, s1 - s0)
    return out_full


# revision 3
# speedup vs baseline: 1.2313x; 1.2313x over previous
"""Trainium2 Bass kernel: CMAFM fusion (segment min/max stats -> attention
MLPs -> gated 2-layer MLP over voxels), data-parallel over the batch axis.

Sharding: batch b -> NeuronCore b (batch_idx is sorted, B == n_cores == 8).
Each core computes its own batch's feature min/max stats locally, runs the
tiny attention MLPs on-device, folds the per-feature gating into the first
fused-MLP weight matrix as a per-partition scale, and runs the big MLP
over its voxels. No collectives: every voxel's gating row is core-local
by construction.

Design notes:
- Ingest via hardware DMA-transpose (xbar): HBM [chunk,128] -> SBUF
  [128,chunk] feature-major directly. Kills all PE transposes, PSUM
  staging, staged copies, and the host-side input permutation.
- Segment min/max via bf16 tensor_tensor accumulate chains on DVE (the
  2x uop tier; tensor_reduce only has a 1x uop) + log-fold at finalize.
  Accumulators init once to +/-1e30 and stay idempotent across reps.
- Fused-MLP layer 2 runs operand-swapped (W_f2 slices stationary as lhsT,
  h1 streams as rhs, N=512) producing feature-major output; the store
  writes [2,128,S] with 2-4KB-contiguous descriptors and the host
  transposes back when unsharding.
- PSUM evacuation split across Act and DVE (PSUM sources cap each engine
  at ~1 elem/cycle/lane); all DMA on the SP HWDGE ring.
Weights load once; the rep loop re-runs ingest + stats + MLP with rep
r+1's ingest interleaved into rep r's MLP loop so consecutive reps
pipeline across engines.
"""

import os
import sys

import numpy as np

for _p in ("/opt/trn_rl_repo",):
    if os.path.isdir(_p) and _p not in sys.path:
        sys.path.append(_p)

B = 8
L = 128
C = 128
OUT = 256
CA = 512
H = 170
VT = 512     # voxels per compute tile
CH = 2048    # voxels per ingest/store chunk (tail chunk may be 1024)

STATS_GP = 0  # retained for older harness compatibility (unused)

_cache = {}


def _chunks(S_pad):
    spans = []
    v0 = 0
    while v0 < S_pad:
        w = min(CH, S_pad - v0)
        spans.append((v0, w))
        v0 += w
    return spans


def _build(S_pad, stats_gp=0, reps=1):
    from contextlib import ExitStack

    import concourse.bacc as bacc
    import concourse.mybir as mybir
    import concourse.tile as tile

    f32 = mybir.dt.float32
    bf16 = mybir.dt.bfloat16
    Alu = mybir.AluOpType
    Act = mybir.ActivationFunctionType

    spans = _chunks(S_pad)
    NCH = len(spans)

    nc = bacc.Bacc("TRN2", target_bir_lowering=False, debug=False, num_devices=B)
    lidar = nc.dram_tensor("lidar", [S_pad, L], bf16, kind="ExternalInput").ap()
    cam = nc.dram_tensor("cam", [S_pad, C], bf16, kind="ExternalInput").ap()
    wl1 = nc.dram_tensor("W_l1", [CA, H], f32, kind="ExternalInput").ap()
    wl2 = nc.dram_tensor("W_l2", [H, L], f32, kind="ExternalInput").ap()
    wc1 = nc.dram_tensor("W_c1", [CA, H], f32, kind="ExternalInput").ap()
    wc2 = nc.dram_tensor("W_c2", [H, C], f32, kind="ExternalInput").ap()
    wf1 = nc.dram_tensor("W_f1", [2 * L, OUT], f32, kind="ExternalInput").ap()
    wf2 = nc.dram_tensor("W_f2", [OUT, OUT], f32, kind="ExternalInput").ap()
    out = nc.dram_tensor("out", [2, 128, S_pad], bf16, kind="ExternalOutput").ap()

    with tile.TileContext(nc) as tc, ExitStack() as ctx:
        wpool = ctx.enter_context(tc.tile_pool(name="weights", bufs=1))
        respool = ctx.enter_context(tc.tile_pool(name="res", bufs=1))
        statpool = ctx.enter_context(tc.tile_pool(name="stat", bufs=1))
        h1pool = ctx.enter_context(tc.tile_pool(name="h1", bufs=2))
        xoutpool = ctx.enter_context(tc.tile_pool(name="xout", bufs=2))
        psl1 = ctx.enter_context(tc.tile_pool(name="psl1", bufs=2, space="PSUM"))
        psl2 = ctx.enter_context(tc.tile_pool(name="psl2", bufs=2, space="PSUM"))

        # --- weight SBUF tiles ---
        wf1_s = wpool.tile([128, 2, OUT], f32)
        wf2_stage = wpool.tile([128, 2, OUT], f32)
        wf2_s = wpool.tile([128, 2, OUT], bf16)
        w1e_s = wpool.tile([128, 2, OUT], bf16)
        wl1_s = wpool.tile([128, 4, H], f32)
        wc1_s = wpool.tile([128, 4, H], f32)
        wl2a_s = wpool.tile([128, L], f32)
        wl2b_s = wpool.tile([H - 128, L], f32)
        wc2a_s = wpool.tile([128, C], f32)
        wc2b_s = wpool.tile([H - 128, C], f32)

        nc.sync.dma_start(wl1_s[:], wl1.rearrange("(a p) h -> p a h", p=128))
        nc.sync.dma_start(wc1_s[:], wc1.rearrange("(a p) h -> p a h", p=128))
        nc.sync.dma_start(wl2a_s[:], wl2[0:128, :])
        nc.sync.dma_start(wl2b_s[:], wl2[128:H, :])
        nc.sync.dma_start(wc2a_s[:], wc2[0:128, :])
        nc.sync.dma_start(wc2b_s[:], wc2[128:H, :])
        nc.sync.dma_start(wf1_s[:], wf1.rearrange("(a p) o -> p a o", p=128))
        nc.sync.dma_start(wf2_stage[:], wf2.rearrange("(a p) o -> p a o", p=128))
        nc.scalar.activation(wf2_s[:], wf2_stage[:], Act.Copy)

        # preload activation tables off the critical path
        warm = wpool.tile([128, 1], f32)
        nc.vector.memset(warm[:], 0.0)
        nc.scalar.activation(warm[:], warm[:], Act.Relu)
        nc.scalar.activation(warm[:], warm[:], Act.Sigmoid)

        # resident feature-major voxel data + per-chunk stat partials
        xres = {
            "l": respool.tile([128, S_pad], bf16, name="xres_l", tag="xres_l"),
            "c": respool.tile([128, S_pad], bf16, name="xres_c", tag="xres_c"),
        }
        stream_src = {"l": lidar, "c": cam}
        accs = {}
        for key in ("min_l", "max_l", "min_c", "max_c"):
            accs[key] = statpool.tile([128, CH], bf16, name="a" + key,
                                      tag="a" + key)
            nc.vector.memset(
                accs[key][:], 1e30 if key.startswith("min") else -1e30
            )

        chain_of = {"min_l": 0, "max_l": 1, "min_c": 2, "max_c": 3}
        tiny_ctx = {}

        def emit_ingest_chunk(ci):
            """Transpose-DMA one chunk of both streams + 4 stat reduces."""
            c0, w = spans[ci]
            for which in ("l", "c"):
                nc.sync.dma_start_transpose(
                    xres[which][:, c0 : c0 + w],
                    stream_src[which][c0 : c0 + w, :],
                )
            for which in ("l", "c"):
                for statname, op in (("min", Alu.min), ("max", Alu.max)):
                    key = statname + "_" + which
                    acc = accs[key][:, 0:w]
                    nc.vector.tensor_tensor(
                        out=acc, in0=acc,
                        in1=xres[which][:, c0 : c0 + w], op=op,
                    )

        def emit_finalize_tiny():
            """Combine partials -> stats -> tiny MLPs -> gated W_f1 fold."""
            ps_t = psl1.tile([128, 2, VT], f32, name="ps1", tag="psl1")
            tiny_all = ps_t[:, 0, 0:OUT]
            for key in ("min_l", "max_l", "min_c", "max_c"):
                k = chain_of[key]
                op = Alu.min if key.startswith("min") else Alu.max
                w = CH
                while w > 128:
                    h = w // 2
                    nc.vector.tensor_tensor(
                        out=accs[key][:, 0:h], in0=accs[key][:, 0:h],
                        in1=accs[key][:, h:w], op=op,
                    )
                    w = h
                s = statpool.tile([128, 1], f32, tag="stat" + key)
                nc.vector.tensor_reduce(
                    s[:], accs[key][:, 0:w], axis=mybir.AxisListType.X, op=op
                )
                for name, w1_s in (("l", wl1_s), ("c", wc1_s)):
                    for tag, mo, mn in (("h1a", 0, 128), ("h1b", 128, H - 128)):
                        ch = 2 * (name == "c") + (tag == "h1b")
                        nc.tensor.matmul(
                            tiny_all[0:mn, ch * 4 + k : ch * 4 + k + 1],
                            w1_s[:, k, mo : mo + mn],
                            s[:],
                            start=True,
                            stop=True,
                        )

            heads = {"l": (wl2a_s, wl2b_s), "c": (wc2a_s, wc2b_s)}
            hsums = {}
            for name in ("l", "c"):
                for tag, mo, mn in (("h1a", 0, 128), ("h1b", 128, H - 128)):
                    ch = 2 * (name == "c") + (tag == "h1b")
                    hp = statpool.tile([mn, 1], f32, tag=tag + "p" + name)
                    nc.vector.tensor_reduce(
                        hp[:], tiny_all[0:mn, ch * 4 : ch * 4 + 4],
                        axis=mybir.AxisListType.X, op=Alu.add,
                    )
                    hs = statpool.tile([mn, 1], f32, tag=tag + "s" + name)
                    nc.vector.tensor_scalar_max(hs[:], hp[:], 0.0)
                    hsums[name, tag] = hs
            for name in ("l", "c"):
                acol = 16 if name == "l" else 18
                w2a_s, w2b_s = heads[name]
                nc.tensor.matmul(
                    tiny_all[:, acol : acol + 1], w2a_s[:],
                    hsums[name, "h1a"][:], start=True, stop=True,
                )
                nc.tensor.matmul(
                    tiny_all[:, acol + 1 : acol + 2], w2b_s[:],
                    hsums[name, "h1b"][:], start=True, stop=True,
                )
            atts = {}
            for name in ("l", "c"):
                acol = 16 if name == "l" else 18
                att_r = statpool.tile([128, 1], f32, tag="attr" + name)
                nc.vector.tensor_reduce(
                    att_r[:], tiny_all[:, acol : acol + 2],
                    axis=mybir.AxisListType.X, op=Alu.add,
                )
                attp = statpool.tile([128, 1], f32, tag="attp" + name)
                nc.vector.tensor_scalar_max(attp[:], att_r[:], 0.0)
                att = statpool.tile([128, 1], f32, tag="att" + name)
                nc.scalar.activation(att[:], attp[:], Act.Sigmoid)
                atts[name] = att
            nc.scalar.activation(
                w1e_s[:, 0, :], wf1_s[:, 0, :], Act.Copy, scale=atts["l"][:]
            )
            nc.scalar.activation(
                w1e_s[:, 1, :], wf1_s[:, 1, :], Act.Copy, scale=atts["c"][:]
            )

        def emit_l1(t):
            """L1 matmuls for tile t -> [128(hid half), 2, VT] PSUM + relu."""
            xt_l = xres["l"][:, t * VT : (t + 1) * VT]
            xt_c = xres["c"][:, t * VT : (t + 1) * VT]
            ps1 = psl1.tile([128, 2, VT], f32, name="ps1", tag="psl1")
            for m in range(2):
                sl = ps1[:, m, :]
                nc.tensor.matmul(
                    sl, w1e_s[:, 0, m * 128 : (m + 1) * 128], xt_l,
                    start=True, stop=False,
                )
                nc.tensor.matmul(
                    sl, w1e_s[:, 1, m * 128 : (m + 1) * 128], xt_c,
                    start=False, stop=True,
                )
            h1t = h1pool.tile([128, 2, VT], bf16, tag="h1")
            nc.scalar.activation(h1t[:], ps1[:], Act.Relu)
            return h1t

        def emit_l2(t, h1t, xout, o):
            """L2 (W_f2 stationary): [128(out half), 2, VT] PSUM + relu."""
            ps2 = psl2.tile([128, 2, VT], f32, name="ps2", tag="psl2")
            for g in range(2):
                sl = ps2[:, g, :]
                nc.tensor.matmul(
                    sl, wf2_s[:, 0, g * 128 : (g + 1) * 128], h1t[:, 0, :],
                    start=True, stop=False,
                )
                nc.tensor.matmul(
                    sl, wf2_s[:, 1, g * 128 : (g + 1) * 128], h1t[:, 1, :],
                    start=False, stop=True,
                )
            dst = xout[:, :, o : o + VT]
            if t % 2 == 0:
                nc.vector.tensor_scalar_max(dst, ps2[:], 0.0)
            else:
                nc.scalar.activation(dst, ps2[:], Act.Relu)

        # prologue: rep 0's ingest + finalize
        for ci in range(NCH):
            emit_ingest_chunk(ci)
        emit_finalize_tiny()

        for r in range(reps):
            h1cur = emit_l1(0)
            tprev = 0
            oprev = None
            for ci in range(NCH):
                c0, w = spans[ci]
                xout = xoutpool.tile([128, 2, CH], bf16, tag="xout")
                for ti in range(w // VT):
                    t = c0 // VT + ti
                    # software pipeline: issue l1(t+1) before l2(t)
                    nxt = t + 1
                    h1next = emit_l1(nxt) if nxt < S_pad // VT else None
                    emit_l2(t, h1cur, xout, ti * VT)
                    h1cur = h1next
                nc.sync.dma_start(
                    out[:, :, c0 : c0 + w].rearrange("g p s -> p g s"),
                    xout[:, :, 0:w],
                )
                if r + 1 < reps:
                    emit_ingest_chunk(ci)
            if r + 1 < reps:
                emit_finalize_tiny()

    nc.compile()
    return nc


def _get_program(S_pad):
    if S_pad not in _cache:
        _cache[S_pad] = _build(S_pad)
    return _cache[S_pad]


def _to_bf16(a):
    import ml_dtypes

    return np.asarray(a, np.float32).astype(ml_dtypes.bfloat16)


def shard_inputs(lidar, cam, batch_idx, W_l1, W_l2, W_c1, W_c2, W_f1, W_f2):
    """Split by batch (batch_idx sorted), pad with replicated real rows."""
    lidar = _to_bf16(lidar)
    cam = _to_bf16(cam)
    batch_idx = np.asarray(batch_idx)
    bounds = np.searchsorted(batch_idx, np.arange(B + 1))
    sizes = np.diff(bounds)
    S_pad = int(-(-max(int(sizes.max()), 1) // 1024) * 1024)
    weights = {
        "W_l1": np.ascontiguousarray(W_l1, np.float32),
        "W_l2": np.ascontiguousarray(W_l2, np.float32),
        "W_c1": np.ascontiguousarray(W_c1, np.float32),
        "W_c2": np.ascontiguousarray(W_c2, np.float32),
        "W_f1": np.ascontiguousarray(W_f1, np.float32),
        "W_f2": np.ascontiguousarray(W_f2, np.float32),
    }
    in_maps = []
    for b in range(B):
        s0, s1 = int(bounds[b]), int(bounds[b + 1])
        n = s1 - s0
        l = np.empty((S_pad, L), lidar.dtype)
        c = np.empty((S_pad, C), cam.dtype)
        if n > 0:
            l[:n] = lidar[s0:s1]
            c[:n] = cam[s0:s1]
            l[n:] = lidar[s1 - 1]
            c[n:] = cam[s1 - 1]
        else:
            l[:] = 0
            c[:] = 0
        in_maps.append({"lidar": l, "cam": c, **weights})
    return in_maps, bounds, sizes, S_pad


def unshard_one(res_out, S_pad, n):
    """[2,128,S_pad] bf16 feature-major -> [n, 256] f32 voxel-major."""
    r = np.asarray(res_out).reshape(2 * 128, S_pad)
    return r[:, :n].T.astype(np.float32)


def kernel(lidar, cam, batch_idx, W_l1, W_l2, W_c1, W_c2, W_f1, W_f2):
    from concourse.bass_utils import run_bass_kernel_spmd

    in_maps, bounds, sizes, S_pad = shard_inputs(
        lidar, cam, batch_idx, W_l1, W_l2, W_c1, W_c2, W_f1, W_f2
    )
    nc = _get_program(S_pad)
    res = run_bass_kernel_spmd(nc, in_maps, core_ids=list(range(B)))
    N = lidar.shape[0]
    out_full = np.empty((N, OUT), np.float32)
    for b in range(B):
        s0, s1 = int(bounds[b]), int(bounds[b + 1])
        if s1 > s0:
            out_full[s0:s1] = unshard_one(res.results[b]["out"], S_pad, s1 - s0)
    return out_full
